# revision 1
# baseline (speedup 1.0000x reference)
"""ContextualLoss forward on 8 Trainium2 NeuronCores.

Math (reference):
    mu[m]   = mean_c Y[c, m]                      (PONO over channels of Y)
    Xc = X - mu ; Yc = Y - mu                     (both centered by Y's mean)
    cos[i,j] = <Xc_i, Yc_j> / (|Xc_i| |Yc_j|)
    d = 1 - cos ; dn = d / (min_j d + 1e-3) ; w = exp((1 - dn)/0.1)
    A = w / sum_j w ; CX_b = mean_i max_j A ; loss = mean_b -log CX_b

Device-side tricks:
  * Only Y is centered explicitly. Since Yc has zero channel-mean,
    <Xc_i, Yc_j> == <X_i, Yc_j>, so raw X feeds the matmul.
  * max_j A = exp-at-dmin / sum_j w = exp(0.01/(dmin+1e-3)) / sum_j w
    (w is monotone decreasing in d) -> no second max pass over w.
  * The per-column scale 1/|Yc_j| is folded into a single fused DVE
    tensor_tensor_reduce: d_sb = G * inv_ny_bcast with a max-accumulator,
    which is also the PSUM->SBUF move.
  * Per-row scale 1/|Xc_i| and the softmin exponent fold into the ScalarE
    activation: w = Exp(scale_i * dsc + bias_i), scale_i = s*inv_nx,
    bias_i = 10 - s, s = 10/(dmin+1e-3); accum_out gives sum_j w for free.

Sharding: core c -> sample b = c//2, row-half h = c%2 (2048 rows each).
Each core's Y is column-permuted host-side to [own-half | other-half] so the
identical SPMD program can read the X-half's means from columns [0, 2048).
Row reductions are permutation-invariant, so the permutation is harmless.
"""

import os
import sys
from contextlib import ExitStack

sys.path.insert(0, "/opt/trn_rl_repo")

import numpy as np

import concourse.bass as bass
import concourse.tile as tile
from concourse import bacc
from concourse import mybir
from concourse.bass_utils import run_bass_kernel_spmd

B = 4
C = 256
M = 4096  # 64*64 spatial positions
HALF = M // 2  # rows per core
NT = HALF // 128  # 16 i-tiles per core
N_CORES = 8

F32 = mybir.dt.float32
F32R = mybir.dt.float32r
BF16 = mybir.dt.bfloat16
AF = mybir.ActivationFunctionType
ALU = mybir.AluOpType

NEG_HUGE = -3.0e38


def _r(ap):
    """View a fp32 AP as float32r for full-rate PE matmul."""
    return ap.bitcast(F32R)


def build_nc() -> bass.Bass:
    nc = bacc.Bacc()

    x_d = nc.declare_dram_parameter("x", [C, HALF], F32, isOutput=False)
    y_d = nc.declare_dram_parameter("y", [C, M], F32, isOutput=False)
    v_d = nc.declare_dram_parameter("v", [128, NT], F32, isOutput=True)

    Q = 1024  # preprocessing quarter width

    with tile.TileContext(nc) as tc:
        with (
            tc.tile_pool(name="io", bufs=1) as io,
            tc.tile_pool(name="consts", bufs=1) as consts,
            tc.tile_pool(name="stats", bufs=1) as stats,
        ):
            # ---- inputs -> SBUF: y half 0 first (feeds the sy/center
            # chain), then x, then y half 1 ------------------------------
            x_bf = io.tile([128, 2, HALF], BF16)
            y_bf = io.tile([128, 2, M], BF16)

            # ---- constants ------------------------------------------------
            ones_col = consts.tile([128, 1], F32)
            nc.vector.memset(ones_col, 1.0)
            ones_col_bf = consts.tile([128, 1], BF16)
            nc.vector.memset(ones_col_bf, 1.0)
            ones_row = consts.tile([1, 128], F32)
            nc.vector.memset(ones_row, 1.0)
            ones_row_bf = consts.tile([1, 128], BF16)
            nc.vector.memset(ones_row_bf, 1.0)
            inv256_row_bf = consts.tile([1, 128], BF16)
            nc.vector.memset(inv256_row_bf, 1.0 / 256.0)
            ten_col = consts.tile([128, 1], F32)
            nc.vector.memset(ten_col, 10.0)
            one_1x1 = consts.tile([1, 1], F32)
            nc.vector.memset(one_1x1, 1.0)
            one_1x1_bf = consts.tile([1, 1], BF16)
            nc.vector.memset(one_1x1_bf, 1.0)

            inv_ny_b = io.tile([128, M], F32)  # |Yc| then 1/|Yc| broadcast

            nx2 = stats.tile([128, NT], F32)
            inv_nx = stats.tile([128, NT], F32)
            inv_nx10 = stats.tile([128, NT], F32)
            r16 = stats.tile([128, NT], F32)
            sumw16 = stats.tile([128, NT], F32)
            maxw16 = stats.tile([128, NT], F32)
            rs16 = stats.tile([128, NT], F32)
            v16 = stats.tile([128, NT], F32)
            t_b = stats.tile([128, NT], F32)

            # main-loop pools opened alongside preprocessing so tile-0
            # quarters can interleave with the tail of preprocessing
            mstack = ExitStack()
            dpool = mstack.enter_context(tc.tile_pool(name="dpool", bufs=4))
            wpool = mstack.enter_context(tc.tile_pool(name="wpool", bufs=1))
            mains = mstack.enter_context(tc.tile_pool(name="mains", bufs=4))
            psum_g = mstack.enter_context(tc.tile_pool(name="psum_g", bufs=2, space="PSUM"))

            d_sb0 = dpool.tile([128, M], F32, tag="d_sb")
            cmax0 = mains.tile([128, 4], F32, tag="cmax2")

            def g_quarter(t, g, d_tile, cmax_tile):
                ps = psum_g.tile([128, Q], F32, tag="g")
                for k in range(2):
                    for j in range(2):
                        nc.tensor.matmul(
                            ps[:, j * 512 : (j + 1) * 512],
                            lhsT=x_bf[:, k, t * 128 : (t + 1) * 128],
                            rhs=y_bf[:, k, g * Q + j * 512 : g * Q + (j + 1) * 512],
                            start=(k == 0),
                            stop=(k == 1),
                        )
                if os.environ.get("USE_TTR", "") != "1":
                    nc.vector.tensor_mul(
                        d_tile[:, g * Q : (g + 1) * Q],
                        ps[:, :],
                        inv_ny_b[:, g * Q : (g + 1) * Q],
                    )
                    nc.vector.reduce_max(
                        cmax_tile[:, g : g + 1],
                        d_tile[:, g * Q : (g + 1) * Q],
                        axis=mybir.AxisListType.X,
                    )
                else:
                    nc.vector.tensor_tensor_reduce(
                        out=d_tile[:, g * Q : (g + 1) * Q],
                        in0=ps[:, :],
                        in1=inv_ny_b[:, g * Q : (g + 1) * Q],
                        scale=1.0,
                        scalar=NEG_HUGE,
                        op0=ALU.mult,
                        op1=ALU.max,
                        accum_out=cmax_tile[:, g : g + 1],
                    )

            with (
                tc.tile_pool(name="psum_pre", bufs=2, space="PSUM") as pre,
                tc.tile_pool(name="rows", bufs=1) as rows,
                tc.tile_pool(name="scratch", bufs=3) as scratch,
            ):
                sy_row = rows.tile([1, M], BF16, tag="rowM")
                qy_row = rows.tile([1, M], BF16, tag="rowM2")
                sq = rows.tile([128, 2, M], BF16)  # squares staging (bf16: only feeds aggregate norms)

                ystage = {}

                y_v = y_d.rearrange("(k p) m -> p k m", p=128)
                x_v = x_d.rearrange("(k p) m -> p k m", p=128)

                def y_dma_quarter(q):
                    st = scratch.tile([128, 2, Q], F32, tag="stage")
                    nc.sync.dma_start(
                        out=st[:, :, :], in_=y_v[:, :, q * Q : (q + 1) * Q]
                    )
                    # raw y -> bf16 (centered in place later)
                    nc.vector.tensor_copy(
                        y_bf[:, :, q * Q : (q + 1) * Q], st[:, :, :]
                    )
                    ystage[q] = st

                def x_quarter(q):
                    st = scratch.tile([128, 2, Q], F32, tag="stage")
                    nc.sync.dma_start(
                        out=st[:, :, :], in_=x_v[:, :, q * Q : (q + 1) * Q]
                    )
                    nc.scalar.copy(
                        x_bf[:, :, q * Q : (q + 1) * Q], st[:, :, :]
                    )

                def xcenter_quarter(q):
                    # x_bf -= mu (in place); then xc^2 -> sq (overwrites y^2
                    # region after qy MMs have consumed it)
                    ps = pre.tile([128, Q], F32, tag="pre")
                    for j in range(2):
                        nc.tensor.matmul(
                            ps[:, j * 512 : (j + 1) * 512],
                            lhsT=inv256_row_bf[:, :],
                            rhs=sy_row[:, q * Q + j * 512 : q * Q + (j + 1) * 512],
                            start=True,
                            stop=True,
                        )
                    for k in range(2):
                        nc.vector.tensor_sub(
                            x_bf[:, k, q * Q : (q + 1) * Q],
                            x_bf[:, k, q * Q : (q + 1) * Q],
                            ps[:, :],
                        )
                    nc.scalar.activation(
                        sq[:, :, q * Q : (q + 1) * Q],
                        x_bf[:, :, q * Q : (q + 1) * Q],
                        AF.Square,
                    )

                def sy_quarter(q):
                    ps = pre.tile([1, Q], F32, tag="pre")
                    for k in range(2):
                        for j in range(2):
                            nc.tensor.matmul(
                                ps[:, j * 512 : (j + 1) * 512],
                                lhsT=ones_col_bf[:, :],
                                rhs=y_bf[:, k, q * Q + j * 512 : q * Q + (j + 1) * 512],
                                start=(k == 0),
                                stop=(k == 1),
                            )
                    nc.scalar.copy(sy_row[:, q * Q : (q + 1) * Q], ps[:, :])

                def center_quarter(q):
                    # mu broadcast (1/256 via lhsT), subtract into bf16, then y^2
                    ps = pre.tile([128, Q], F32, tag="pre")
                    for j in range(2):
                        nc.tensor.matmul(
                            ps[:, j * 512 : (j + 1) * 512],
                            lhsT=inv256_row_bf[:, :],
                            rhs=sy_row[:, q * Q + j * 512 : q * Q + (j + 1) * 512],
                            start=True,
                            stop=True,
                        )
                    for k in range(2):
                        nc.vector.tensor_sub(
                            y_bf[:, k, q * Q : (q + 1) * Q],
                            y_bf[:, k, q * Q : (q + 1) * Q],
                            ps[:, :],
                        )
                    nc.scalar.activation(
                        sq[:, :, q * Q : (q + 1) * Q],
                        y_bf[:, :, q * Q : (q + 1) * Q],
                        AF.Square,
                    )

                def qy_quarter(q):
                    ps = pre.tile([1, Q], F32, tag="pre")
                    for k in range(2):
                        for j in range(2):
                            nc.tensor.matmul(
                                ps[:, j * 512 : (j + 1) * 512],
                                lhsT=ones_col_bf[:, :],
                                rhs=sq[:, k, q * Q + j * 512 : q * Q + (j + 1) * 512],
                                start=(k == 0),
                                stop=(k == 1),
                            )
                    nc.scalar.copy(qy_row[:, q * Q : (q + 1) * Q], ps[:, :])

                def invb_quarter(q):
                    ps = pre.tile([128, Q], F32, tag="pre")
                    for j in range(2):
                        nc.tensor.matmul(
                            ps[:, j * 512 : (j + 1) * 512],
                            lhsT=ones_row_bf[:, :],
                            rhs=qy_row[:, q * Q + j * 512 : q * Q + (j + 1) * 512],
                            start=True,
                            stop=True,
                        )
                    nc.scalar.activation(
                        inv_ny_b[:, q * Q : (q + 1) * Q], ps[:, :], AF.Sqrt
                    )
                    nc.vector.reciprocal(
                        inv_ny_b[:, q * Q : (q + 1) * Q],
                        inv_ny_b[:, q * Q : (q + 1) * Q],
                    )


                def stat16(dst16, src_tile):
                    # dst16[p, t] = sum_c src[c, t*128+p] via N=1 matmuls
                    ps = pre.tile([128, NT], F32, tag="pre")
                    for t in range(NT):
                        for k in range(2):
                            nc.tensor.matmul(
                                ps[:, t : t + 1],
                                lhsT=src_tile[:, k, t * 128 : (t + 1) * 128],
                                rhs=ones_col_bf[:, :],
                                start=(k == 0),
                                stop=(k == 1),
                            )
                    nc.vector.tensor_copy(dst16[:, :], ps[:, :])

                # ---- phase schedule (program order ~ priority) ----------
                y_dma_quarter(0)
                y_dma_quarter(1)
                sy_quarter(0)
                sy_quarter(1)
                center_quarter(0)
                center_quarter(1)
                qy_quarter(0)
                invb_quarter(0)
                qy_quarter(1)
                invb_quarter(1)
                x_quarter(0)
                x_quarter(1)
                xcenter_quarter(0)
                xcenter_quarter(1)
                if os.environ.get("BISECT", "") != "pre":
                    g_quarter(0, 0, d_sb0, cmax0)
                    g_quarter(0, 1, d_sb0, cmax0)
                stat16(nx2, sq)
                # inv_nx from nx2 (already tile-major)
                nc.scalar.activation(t_b[:, :], nx2[:, :], AF.Sqrt)
                nc.vector.reciprocal(inv_nx[:, :], t_b[:, :])
                nc.vector.tensor_scalar_mul(inv_nx10[:, :], inv_nx[:, :], 10.0)
                y_dma_quarter(2)
                y_dma_quarter(3)
                sy_quarter(2)
                sy_quarter(3)
                center_quarter(2)
                center_quarter(3)
                qy_quarter(2)
                invb_quarter(2)
                qy_quarter(3)
                invb_quarter(3)
                if os.environ.get("BISECT", "") != "pre":
                    g_quarter(0, 2, d_sb0, cmax0)
                    g_quarter(0, 3, d_sb0, cmax0)


            # ---- main loop (pools opened above; t=0 quarters already
            # issued inside preprocessing) --------------------------------
            for t in range([0, NT][os.environ.get("BISECT", "") != "pre"]):
                if t == 0:
                    d_sb, cmax2 = d_sb0, cmax0
                else:
                    d_sb = dpool.tile([128, M], F32, tag="d_sb")
                    cmax2 = mains.tile([128, 4], F32, tag="cmax2")
                    for g in range(4):
                        g_quarter(t, g, d_sb, cmax2)
                cmax = mains.tile([128, 1], F32)
                u = mains.tile([128, 1], F32)
                bias_i = mains.tile([128, 1], F32)
                scale_i = mains.tile([128, 1], F32)
                nc.vector.reduce_max(cmax[:, :], cmax2[:, :], axis=mybir.AxisListType.X)
                nc.vector.tensor_mul(cmax[:, :], cmax[:, :], inv_nx[:, t : t + 1])
                # u = dmin + 1e-3 = 1.001 - cosmax
                nc.vector.tensor_scalar(
                    out=u[:, :],
                    in0=cmax[:, :],
                    scalar1=-1.0,
                    scalar2=1.001,
                    op0=ALU.mult,
                    op1=ALU.add,
                )
                nc.vector.reciprocal(r16[:, t : t + 1], u[:, :])
                # scale = 10*r*inv_nx ; bias = 10 - 10*r   (on ScalarE)
                nc.scalar.activation(
                    scale_i[:, :], r16[:, t : t + 1], AF.Identity,
                    scale=inv_nx10[:, t : t + 1],
                )
                nc.scalar.activation(
                    bias_i[:, :], r16[:, t : t + 1], AF.Identity,
                    scale=-10.0, bias=ten_col[:, :],
                )
                w_sb = wpool.tile([128, M], BF16)
                nc.scalar.activation(
                    out=w_sb[:, :],
                    in_=d_sb[:, :],
                    func=AF.Exp,
                    bias=bias_i[:, :],
                    scale=scale_i[:, :],
                    accum_out=sumw16[:, t : t + 1],
                )

            # ---- epilogue: v = exp(0.01*r) / sumw -----------------------
            if os.environ.get("BISECT", "") == "pre":
                nc.vector.tensor_copy(v16[:, :], inv_nx[:, :])
            else:
                nc.scalar.activation(maxw16[:, :], r16[:, :], AF.Exp, scale=0.01)
                nc.vector.reciprocal(rs16[:, :], sumw16[:, :])
                nc.vector.tensor_mul(v16[:, :], maxw16[:, :], rs16[:, :])
            nc.sync.dma_start(out=v_d[:, :], in_=v16[:, :])

            mstack.close()

    nc.compile()
    return nc

_NC = None


def _get_nc():
    global _NC
    if _NC is None:
        _NC = build_nc()
    return _NC


def make_in_maps(X, Y):
    """Per-core inputs. Y columns permuted to [own-half | other-half]."""
    in_maps = []
    for c in range(N_CORES):
        b, h = c // 2, c % 2
        xs = np.ascontiguousarray(X[b][:, h * HALF : (h + 1) * HALF])
        ys = np.ascontiguousarray(
            np.concatenate(
                [
                    Y[b][:, h * HALF : (h + 1) * HALF],
                    Y[b][:, (1 - h) * HALF : (2 - h) * HALF],
                ],
                axis=1,
            )
        )
        in_maps.append({"x": xs, "y": ys})
    return in_maps


def finish_host(results):
    """results: list of 8 per-core dicts with 'v' [128, NT]."""
    cx = np.zeros(B, dtype=np.float64)
    for c in range(N_CORES):
        cx[c // 2] += results[c]["v"].astype(np.float64).sum()
    cx /= M
    return np.float32(np.mean(-np.log(cx)))


def run(X_features, Y_features, trace=False, tmpdir=None):
    X = np.asarray(X_features, dtype=np.float32).reshape(B, C, M)
    Y = np.asarray(Y_features, dtype=np.float32).reshape(B, C, M)
    nc = _get_nc()
    res = run_bass_kernel_spmd(
        nc, make_in_maps(X, Y), list(range(N_CORES)), trace=trace, tmpdir=tmpdir
    )
    return finish_host(res.results), res


def kernel(X_features, Y_features):
    loss, _ = run(X_features, Y_features)
    return loss



# revision 9
# speedup vs baseline: 1.5676x; 1.5676x over previous
"""ContextualLoss forward on 8 Trainium2 NeuronCores (v3).

Math (reference):
    mu[m]   = mean_c Y[c, m]                      (PONO over channels of Y)
    Xc = X - mu ; Yc = Y - mu                     (both centered by Y's mean)
    cos[i,j] = <Xc_i, Yc_j> / (|Xc_i| |Yc_j|)
    d = 1 - cos ; dn = d / (min_j d + 1e-3) ; w = exp((1 - dn)/0.1)
    A = w / sum_j w ; CX_b = mean_i max_j A ; loss = mean_b -log CX_b

Device-side structure (per core: one sample b, one 2048-row half):
  * Inputs are converted to fp16 on the host: halves DMA traffic and fp16
    matmuls run at full PE rate.
  * Y is centered and column-scaled in place: Yn = (Y - mu) / |Yc|, so the
    big matmul produces G[i,j] = cos[i,j] * |Xc_i| directly in PSUM.
    (X never needs centering for the matmul: Yn has zero channel-sum.)
  * mu and |Yc|^2 are broadcast to [128, M] via "fat ones" matmuls
    (lhsT = 1/256 resp. 1 in every entry), skipping row-copy round trips.
  * |Xc_i|^2 = QX_i - sy_i*SX_i/128 + sy_i^2/256 from per-row raw stats
    (tile-major [128,16] via tiny N=1 matmuls), so X is never modified.
  * The drain DVE tensor_scalar(op0=mult 1.0, op1=max, accum_out) moves each
    PSUM half-tile to fp16 SBUF and computes the row max in one pass.
  * ScalarE does only Exp over the drained d tiles (scale_i, bias_i fold the
    1/|Xc_i| row scale and the softmin exponent; accum_out gives sum_j w).
  * max_j A = exp(0.01/(dmin+1e-3)) / sum_j w analytically (no second pass).

Engine budget per core: PE ~62us (matmuls), DVE ~75us (drain+max scans),
Act ~65us (exp), Pool ~45us (elementwise prep), DMA ~9us (fp16 inputs).

Sharding: core c -> sample b = c//2, row-half h = c%2 (2048 rows each).
Each core's Y is column-permuted host-side to [own-half | other-half] so the
identical SPMD program can read the X-half's stats from columns [0, 2048).
Row reductions are permutation-invariant, so the permutation is harmless.
"""

import sys

sys.path.insert(0, "/opt/trn_rl_repo")

import numpy as np

import concourse.bass as bass
import concourse.tile as tile
from concourse import bacc
from concourse import mybir
from concourse.bass_utils import run_bass_kernel_spmd

B = 4
C = 256
M = 4096  # 64*64 spatial positions
HALF = M // 2  # rows per core
NT = HALF // 128  # 16 i-tiles per core
N_CORES = 8
Q = 1024  # preprocessing quarter width
HW = 2048  # main-loop psum half width

F32 = mybir.dt.float32
F16 = mybir.dt.float16
AF = mybir.ActivationFunctionType
ALU = mybir.AluOpType


def build_nc() -> bass.Bass:
    nc = bacc.Bacc()

    x_d = nc.declare_dram_parameter("x", [C, HALF], F16, isOutput=False)
    y_d = nc.declare_dram_parameter("y", [C, M], F16, isOutput=False)
    v_d = nc.declare_dram_parameter("v", [128, NT], F32, isOutput=True)

    x_v = x_d.rearrange("(k p) m -> p k m", p=128)
    y_v = y_d.rearrange("(k p) m -> p k m", p=128)

    with tile.TileContext(nc) as tc:
        with (
            tc.tile_pool(name="io", bufs=1) as io,
            tc.tile_pool(name="consts", bufs=1) as consts,
            tc.tile_pool(name="stats", bufs=1) as stats,
            tc.tile_pool(name="dpool", bufs=6) as dpool,
            tc.tile_pool(name="wpool", bufs=1) as wpool,
        ):
            y_sb = io.tile([128, 2, M], F16)
            x_sb = io.tile([128, 2, HALF], F16)

            ones_mat = consts.tile([128, 128], F16)
            nc.vector.memset(ones_mat, 1.0)
            inv256_mat = consts.tile([128, 128], F16)
            nc.vector.memset(inv256_mat, 1.0 / 256.0)
            ones_col = consts.tile([128, 1], F16)
            nc.vector.memset(ones_col, 1.0)

            sy16 = stats.tile([128, NT], F32)
            sx16 = stats.tile([128, NT], F32)
            qx16 = stats.tile([128, NT], F32)
            nx2 = stats.tile([128, NT], F32)
            inv_nx = stats.tile([128, NT], F32)
            inv_nx10 = stats.tile([128, NT], F32)
            cmaxA = stats.tile([128, NT], F32)
            cmaxB = stats.tile([128, NT], F32)
            cm16 = stats.tile([128, NT], F32)
            u16 = stats.tile([128, NT], F32)
            r16 = stats.tile([128, NT], F32)
            scale16 = stats.tile([128, NT], F32)
            bias16 = stats.tile([128, NT], F32)
            sumw16 = stats.tile([128, NT], F32)
            maxw16 = stats.tile([128, NT], F32)
            rs16 = stats.tile([128, NT], F32)
            v16 = stats.tile([128, NT], F32)

            # ---------------- preprocessing ----------------
            with (
                tc.tile_pool(name="psum_pre", bufs=2, space="PSUM") as pre,
                tc.tile_pool(name="psum_stat", bufs=2, space="PSUM") as pst,
                tc.tile_pool(name="presb", bufs=1) as presb,
                tc.tile_pool(name="sqpool", bufs=2) as sqpool,
            ):
                mu_b = presb.tile([128, M], F16)
                inv_ny = presb.tile([128, M], F16)
                sqx = presb.tile([128, 2, HALF], F16)

                def fat_mm(ps, lhsT, src, q):
                    # ps[p, j] = sum_c lhsT[c, p] * src[c, j] over both k chunks
                    for k in range(2):
                        for j in range(2):
                            nc.tensor.matmul(
                                ps[:, j * 512 : (j + 1) * 512],
                                lhsT=lhsT[:, :],
                                rhs=src[:, k, q * Q + j * 512 : q * Q + (j + 1) * 512],
                                start=(k == 0),
                                stop=(k == 1),
                            )

                def stat16(dst, src_tile):
                    # dst[p, t] = sum_c src[c, t*128+p] via N=1 matmuls
                    ps = pst.tile([128, NT], F32, tag="stat")
                    for t in range(NT):
                        for k in range(2):
                            nc.tensor.matmul(
                                ps[:, t : t + 1],
                                lhsT=src_tile[:, k, t * 128 : (t + 1) * 128],
                                rhs=ones_col[:, :],
                                start=(k == 0),
                                stop=(k == 1),
                            )
                    nc.vector.tensor_copy(dst[:, :], ps[:, :])

                def mu_quarter(q):
                    # mu broadcast to all partitions + fp16 copy to SBUF
                    ps = pre.tile([128, Q], F32, tag="pre")
                    fat_mm(ps, inv256_mat, y_sb, q)
                    nc.scalar.copy(mu_b[:, q * Q : (q + 1) * Q], ps[:, :])

                def sub_quarter(q):
                    # fp16 SBUF TT -> DVE 2x mode; sits in DVE's pre window
                    for k in range(2):
                        nc.vector.tensor_sub(
                            y_sb[:, k, q * Q : (q + 1) * Q],
                            y_sb[:, k, q * Q : (q + 1) * Q],
                            mu_b[:, q * Q : (q + 1) * Q],
                        )

                def sq_quarter(q):
                    sq = sqpool.tile([128, 2, Q], F16, tag="sq")
                    for k in range(2):
                        nc.vector.tensor_mul(
                            sq[:, k, :],
                            y_sb[:, k, q * Q : (q + 1) * Q],
                            y_sb[:, k, q * Q : (q + 1) * Q],
                        )
                    return sq

                def qy_quarter(q, sq):
                    # |Yc_j|^2 broadcast, then inv_ny = rsqrt -> fp16
                    ps = pre.tile([128, Q], F32, tag="pre")
                    for k in range(2):
                        for j in range(2):
                            nc.tensor.matmul(
                                ps[:, j * 512 : (j + 1) * 512],
                                lhsT=ones_mat[:, :],
                                rhs=sq[:, k, j * 512 : (j + 1) * 512],
                                start=(k == 0),
                                stop=(k == 1),
                            )
                    nc.scalar.activation(
                        inv_ny[:, q * Q : (q + 1) * Q], ps[:, :],
                        AF.Abs_reciprocal_sqrt,
                    )

                def mul_quarter(q):
                    for k in range(2):
                        nc.vector.tensor_mul(
                            y_sb[:, k, q * Q : (q + 1) * Q],
                            y_sb[:, k, q * Q : (q + 1) * Q],
                            inv_ny[:, q * Q : (q + 1) * Q],
                        )

                # DMA order: y half A, x, y half B
                nc.sync.dma_start(out=y_sb[:, :, 0:HW], in_=y_v[:, :, 0:HW])
                nc.sync.dma_start(out=x_sb[:, :, :], in_=x_v[:, :, :])
                nc.sync.dma_start(out=y_sb[:, :, HW:M], in_=y_v[:, :, HW:M])

                # y chain, half A (quarters 0,1); sy16 reads raw y first
                mu_quarter(0)
                mu_quarter(1)
                stat16(sy16, y_sb)  # t-loop reads only columns [0, 2048)
                sub_quarter(0)
                sq0 = sq_quarter(0)
                qy_quarter(0, sq0)
                sub_quarter(1)
                sq1 = sq_quarter(1)
                qy_quarter(1, sq1)
                mul_quarter(0)
                mul_quarter(1)

                # x stats (raw x, raw-y row sums): no x centering needed
                for k in range(2):
                    nc.gpsimd.tensor_mul(
                        sqx[:, k, :], x_sb[:, k, :], x_sb[:, k, :]
                    )
                stat16(sx16, x_sb)
                stat16(qx16, sqx)
                # nx2 = qx - sy*sx/128 + sy^2/256  (all on Pool, off the path)
                t1 = stats.tile([128, NT], F32)
                t2 = stats.tile([128, NT], F32)
                nc.gpsimd.tensor_scalar_mul(t1[:, :], sy16[:, :], -1.0 / 128.0)
                nc.gpsimd.tensor_mul(t1[:, :], t1[:, :], sx16[:, :])
                nc.gpsimd.tensor_add(nx2[:, :], qx16[:, :], t1[:, :])
                nc.gpsimd.tensor_scalar_mul(t2[:, :], sy16[:, :], 1.0 / 256.0)
                nc.gpsimd.tensor_mul(t2[:, :], t2[:, :], sy16[:, :])
                nc.gpsimd.tensor_add(nx2[:, :], nx2[:, :], t2[:, :])
                nc.scalar.activation(inv_nx[:, :], nx2[:, :], AF.Abs_reciprocal_sqrt)
                nc.gpsimd.tensor_scalar_mul(inv_nx10[:, :], inv_nx[:, :], 10.0)

                # y chain, half B (quarters 2,3)
                mu_quarter(2)
                sub_quarter(2)
                sq2 = sq_quarter(2)
                qy_quarter(2, sq2)
                mu_quarter(3)
                sub_quarter(3)
                sq3 = sq_quarter(3)
                qy_quarter(3, sq3)
                mul_quarter(2)
                mul_quarter(3)

            # ---------------- main loop ----------------
            with tc.tile_pool(name="psum_g", bufs=2, space="PSUM") as psg:
                w_sb = wpool.tile([128, M], F16)

                def stats_batch(b):
                    # small per-4-tile ops: Pool except the DVE-only reciprocal
                    sl = slice(4 * b, 4 * b + 4)
                    nc.vector.tensor_tensor(
                        out=cm16[:, sl], in0=cmaxA[:, sl], in1=cmaxB[:, sl],
                        op=ALU.max,
                    )
                    nc.gpsimd.tensor_mul(cm16[:, sl], cm16[:, sl], inv_nx[:, sl])
                    # u = 1.001 - cosmax
                    nc.gpsimd.tensor_scalar(
                        out=u16[:, sl], in0=cm16[:, sl],
                        scalar1=-1.0, scalar2=1.001, op0=ALU.mult, op1=ALU.add,
                    )
                    nc.vector.reciprocal(r16[:, sl], u16[:, sl])
                    nc.gpsimd.tensor_mul(scale16[:, sl], r16[:, sl], inv_nx10[:, sl])
                    nc.gpsimd.tensor_scalar(
                        out=bias16[:, sl], in0=r16[:, sl],
                        scalar1=-10.0, scalar2=10.0, op0=ALU.mult, op1=ALU.add,
                    )

                def exp_tile(t, d_t):
                    nc.scalar.activation(
                        out=w_sb[:, :],
                        in_=d_t[:, :],
                        func=AF.Exp,
                        bias=bias16[:, t : t + 1],
                        scale=scale16[:, t : t + 1],
                        accum_out=sumw16[:, t : t + 1],
                    )

                d_tiles = {}
                for t in range(NT):
                    d_t = dpool.tile([128, M], F16, tag="d")
                    d_tiles[t] = d_t
                    for h in range(2):
                        ps = psg.tile([128, HW], F32, tag="g")
                        for k in range(2):
                            for j in range(4):
                                nc.tensor.matmul(
                                    ps[:, j * 512 : (j + 1) * 512],
                                    lhsT=x_sb[:, k, t * 128 : (t + 1) * 128],
                                    rhs=y_sb[:, k, h * HW + j * 512 : h * HW + (j + 1) * 512],
                                    start=(k == 0),
                                    stop=(k == 1),
                                )
                        cm = cmaxA if h == 0 else cmaxB
                        nc.vector.tensor_scalar(
                            out=d_t[:, h * HW : (h + 1) * HW],
                            in0=ps[:, :],
                            scalar1=1.0,
                            scalar2=None,
                            op0=ALU.mult,
                            op1=ALU.max,
                            accum_out=cm[:, t : t + 1],
                        )
                    if t % 4 == 3:
                        stats_batch(t // 4)
                        for tt in range(t - 3, t + 1):
                            exp_tile(tt, d_tiles.pop(tt))

                # ---- epilogue: v = exp(0.01*r) / sumw ------------------
                nc.scalar.activation(maxw16[:, :], r16[:, :], AF.Exp, scale=0.01)
                nc.vector.reciprocal(rs16[:, :], sumw16[:, :])
                nc.gpsimd.tensor_mul(v16[:, :], maxw16[:, :], rs16[:, :])
                nc.sync.dma_start(out=v_d[:, :], in_=v16[:, :])

    nc.compile()
    return nc


_NC = None


def _get_nc():
    global _NC
    if _NC is None:
        _NC = build_nc()
    return _NC


def make_in_maps(X, Y):
    """Per-core fp16 inputs. Y columns permuted to [own-half | other-half]."""
    in_maps = []
    for c in range(N_CORES):
        b, h = c // 2, c % 2
        xs = np.ascontiguousarray(X[b][:, h * HALF : (h + 1) * HALF]).astype(
            np.float16
        )
        ys = np.ascontiguousarray(
            np.concatenate(
                [
                    Y[b][:, h * HALF : (h + 1) * HALF],
                    Y[b][:, (1 - h) * HALF : (2 - h) * HALF],
                ],
                axis=1,
            )
        ).astype(np.float16)
        in_maps.append({"x": xs, "y": ys})
    return in_maps


def finish_host(results):
    """results: list of 8 per-core dicts with 'v' [128, NT]."""
    cx = np.zeros(B, dtype=np.float64)
    for c in range(N_CORES):
        cx[c // 2] += results[c]["v"].astype(np.float64).sum()
    cx /= M
    return np.float32(np.mean(-np.log(cx)))


def run(X_features, Y_features, trace=False, tmpdir=None):
    X = np.asarray(X_features, dtype=np.float32).reshape(B, C, M)
    Y = np.asarray(Y_features, dtype=np.float32).reshape(B, C, M)
    nc = _get_nc()
    res = run_bass_kernel_spmd(
        nc, make_in_maps(X, Y), list(range(N_CORES)), trace=trace, tmpdir=tmpdir
    )
    return finish_host(res.results), res


def kernel(X_features, Y_features):
    loss, _ = run(X_features, Y_features)
    return loss
